# revision 25
# baseline (speedup 1.0000x reference)
"""Holt-Winters exponential smoothing (level/trend/seasonal, P=7) on 8 Trainium2
NeuronCores.

Math: the per-row recurrence is linear in a 9-dim state
s = [level, trend, buf_0..buf_6]:  s_t = A_{t%7} s_{t-1} + c_{t%7} x_t.
Steps t=1..4095 are processed in 117 chunks of C=35 steps (35 % 7 == 0 so every
chunk sees the same slot pattern and shares one coefficient set), grouped into
9 groups of G=13 chunks.  Chunk-entry states sigma_i come from a per-group
prefix-scan matmul; per chunk the outputs are one K=53 matmul
  Y_c (105,B) = [Wm; U; U].T @ [X_c; sig_hi; sig_lo].

Precision: x and all stationary weights are single bf16 (rel err ~2^-9, small
relative to the 2e-2 gate); only the group-to-group state chain keeps a hi/lo
bf16 split (ws1 hi/lo x state hi/lo, dropping lo*lo) since chain error
compounds over the 9 sequential groups.

Layout: chunks are paired into 128-partition tiles -- pair j holds chunk 2j at
partitions 0:35 (sigma at 35:53) and chunk 2j+1 at 64:99 (sigma at 99:117) --
so one scan matmul covers two chunks and every group's x loads with 2 fat DMAs
that together span both halves of the partition space (all 16 DMA engines).
Outputs stage into one (105, 13*1024) f32 tile per group -> 1 store DMA with
52 KiB contiguous runs per partition.

Sharding: pure data-parallel over the batch axis (1024 rows per core).
"""

import numpy as np

P = 7
C = 35            # chunk size (steps); 35 % 7 == 0
G = 13            # chunks per group
NG = 9            # groups; NG*G*C == L-1
NPAIR = 7         # chunk pairs per group (last pair has only the even chunk)
L = 4096
B = 8192
NCORES = 8
BL = B // NCORES  # 1024 batch rows per core
NHALF = 512       # matmul moving-dim tile (fp32 PSUM bank limit)


def _sigmoid(z):
    return 1.0 / (1.0 + np.exp(-z))


def _step_mats(a, b, g):
    """A_i (9x9), c_i (9,) for seasonal slot i, float64."""
    A, c = [], []
    for i in range(P):
        col = 2 + i
        Ai = np.zeros((9, 9), np.float64)
        ci = np.zeros(9, np.float64)
        Ai[0, 0] = 1 - a
        Ai[0, 1] = 1 - a
        Ai[0, col] += -a
        Ai[1, 0] = -a * b
        Ai[1, 1] = 1 - a * b
        Ai[1, col] += -a * b
        for j in range(P):
            Ai[2 + j, 2 + j] = 1.0
        Ai[col, :] = 0.0
        Ai[col, 0] = -g * (1 - a)
        Ai[col, 1] = -g * (1 - a)
        Ai[col, col] = g * a + 1 - g
        ci[0] = a
        ci[1] = a * b
        ci[col] = g * (1 - a)
        A.append(Ai)
        c.append(ci)
    return A, c


def _hi_lo(x):
    import ml_dtypes
    hi = x.astype(np.float32).astype(ml_dtypes.bfloat16)
    lo = (x.astype(np.float32) - hi.astype(np.float32)).astype(ml_dtypes.bfloat16)
    return hi, lo


def _pcol(j, r):
    """Scan-output column for component r of sigma_j.

    Component-major (r-major) layout within each parity block so the sigma
    scatter's DMA source is a plain contiguous partition range:
      sigma_even[r of pair jp] -> col 7*r + jp        (cols 0:63)
      sigma_odd [r of pair jp] -> col 63 + 6*r + jp   (cols 63:117)
      s_next[r]                -> col 117 + r
    """
    if j == G:
        return 117 + r
    jp, odd = divmod(j, 2)
    return (63 + 6 * r + jp) if odd else (7 * r + jp)


def _build_coeffs(alpha, beta, gamma):
    """Host-precomputed stationary matrices.

    wqp  (NPAIR, 128, 126) bf16: scan lhsT per chunk pair (rows 0:35 even
         chunk's X coeffs, 64:99 odd's; zeros elsewhere)
    ws1h/ws1l (126, 126) bf16: state-propagation lhsT hi/lo
    wmu  (128, 105) bf16: pass-2 lhsT [Wm; U; U] at rows 0:53 and 64:117
    winit (7, 126) f32: init matmul -> s_0 at rows 117:126
    Scan-output columns are permuted: sigma_even at 0:63, sigma_odd at 63:117,
    s_next at 117:126 (so the sigma scatter DMA is 2 dense APs).
    """
    import ml_dtypes
    bf = ml_dtypes.bfloat16
    a, b, g = _sigmoid(alpha), _sigmoid(beta), _sigmoid(gamma)
    A, c = _step_mats(a, b, g)
    slots = [(1 + k) % P for k in range(C)]

    Phi = np.zeros((C, 9, 9), np.float64)
    w = np.zeros((C, C, 9), np.float64)
    cur = np.eye(9)
    for k in range(C):
        i = slots[k]
        if k > 0:
            w[k, :k] = w[k - 1, :k] @ A[i].T
        w[k, k] = c[i]
        cur = A[i] @ cur
        Phi[k] = cur
    T = Phi[C - 1]
    V = w[C - 1].T.copy()  # (9, C)

    Wm = np.zeros((C, 105), np.float64)   # X-coefficient block of pass-2 lhsT
    U = np.zeros((9, 105), np.float64)    # sigma-coefficient block
    for k in range(C):
        sel = [0, 1, 2 + slots[k]]
        U[:, 3 * k:3 * k + 3] = Phi[k][sel].T
        for j in range(k + 1):
            Wm[j, 3 * k:3 * k + 3] = w[k, j][sel]

    Tpow = [np.eye(9)]
    for _ in range(G + 1):
        Tpow.append(T @ Tpow[-1])

    ws1 = np.zeros((126, 126), np.float64)
    for j in range(G + 1):
        for r in range(9):
            ws1[117:126, _pcol(j, r)] = Tpow[j][r, :]
    wqv = np.zeros((G, C, 126), np.float64)
    for i in range(G):
        for j in range(i + 1, G + 1):
            TV = Tpow[j - 1 - i] @ V          # (9, C)
            for r in range(9):
                wqv[i, :, _pcol(j, r)] = TV[r, :]

    winit = np.zeros((7, 126), np.float64)
    winit[0, 117] = 1.0
    winit[0, 118] = -1.0
    winit[1, 118] = 1.0
    for j in range(P):
        winit[j, 119 + j] += 1.0
        winit[0, 119 + j] += -1.0

    ws1_hi, ws1_lo = _hi_lo(ws1)

    wqp = np.zeros((NPAIR, 128, 126), bf)
    for jp in range(NPAIR):
        wqp[jp, 0:C] = wqv[2 * jp].astype(bf)
        if 2 * jp + 1 < G:
            wqp[jp, 64:64 + C] = wqv[2 * jp + 1].astype(bf)

    wmu = np.zeros((128, 105), bf)
    blk = np.concatenate([Wm, U, U], axis=0).astype(bf)   # (53, 105)
    wmu[0:53] = blk
    wmu[64:117] = blk

    return dict(wqp=wqp, ws1h=ws1_hi, ws1l=ws1_lo, wmu=wmu,
                winit=winit.astype(np.float32))


def build_bass(bl=BL):
    """Build the per-core Bass module (SPMD: same module, sharded inputs)."""
    import concourse.bacc as bacc
    import concourse.mybir as mybir
    from concourse.tile import TileContext

    BF = mybir.dt.bfloat16
    F32 = mybir.dt.float32
    nhalf = min(NHALF, bl)
    nh = (bl + nhalf - 1) // nhalf

    nc = bacc.Bacc(None, target_bir_lowering=False, debug=False)
    xb_d = nc.declare_dram_parameter("xb", [NG, 2, C, NPAIR, bl], BF,
                                     isOutput=False)
    x0_d = nc.declare_dram_parameter("x0", [7, bl], F32, isOutput=False)
    wqp_d = nc.declare_dram_parameter("wqp", [NPAIR, 128, 126], BF,
                                      isOutput=False)
    ws1h_d = nc.declare_dram_parameter("ws1h", [126, 126], BF, isOutput=False)
    ws1l_d = nc.declare_dram_parameter("ws1l", [126, 126], BF, isOutput=False)
    wmu_d = nc.declare_dram_parameter("wmu", [128, 105], BF, isOutput=False)
    winit_d = nc.declare_dram_parameter("winit", [7, 126], F32, isOutput=False)
    out_d = nc.declare_dram_parameter("out", [105, NG * G, bl], F32,
                                      isOutput=True)

    from concourse.tile_rust import add_dep_helper as _adh

    def add_dep_helper(frm, to, sync=True, reason=""):
        frm = getattr(frm, "ins", frm)
        to = getattr(to, "ins", to)
        _adh(frm, to, sync, reason)

    with TileContext(nc) as tc:
        with (
            tc.tile_pool(name="consts", bufs=1) as consts,
            tc.tile_pool(name="xpool", bufs=4) as xpool,
            tc.tile_pool(name="spool", bufs=3) as spool,
            tc.tile_pool(name="tpool", bufs=2) as tpool,
            tc.tile_pool(name="ypool", bufs=2) as ypool,
            tc.tile_pool(name="ypsum", bufs=3, space="PSUM") as ypsum,
            tc.tile_pool(name="spsum", bufs=1, space="PSUM") as spsum,
        ):
            wqp = consts.tile([128, NPAIR * 126], BF)
            for j in range(NPAIR):
                nc.sync.dma_start(out=wqp[:, j * 126:(j + 1) * 126],
                                  in_=wqp_d[j])
            ws1h = consts.tile([126, 126], BF)
            nc.sync.dma_start(out=ws1h[:], in_=ws1h_d[:])
            ws1l = consts.tile([126, 126], BF)
            nc.sync.dma_start(out=ws1l[:], in_=ws1l_d[:])
            wmu = consts.tile([128, 105], BF)
            nc.scalar.dma_start(out=wmu[:], in_=wmu_d[:])
            winit = consts.tile([7, 126], F32)
            nc.scalar.dma_start(out=winit[:], in_=winit_d[:])
            xinit = consts.tile([7, bl], F32)
            nc.scalar.dma_start(out=xinit[:], in_=x0_d[:])

            def load_group(g_):
                """x load for group g_: memset NaN-guard rows, then 2 fat
                DMAs (parts 0:35 even band, 64:99 odd band)."""
                xt = xpool.tile([128, NPAIR * bl], BF, tag="xg")
                # zero sigma+pad rows 32:64 (rows 53:64 are read by the scan
                # with zero weights and never DMA-written; must be finite)
                ms = nc.gpsimd.memset(xt[32:64, :], 0.0)
                d1 = nc.sync.dma_start(
                    out=xt[0:C, :].rearrange("p (j c) -> p j c", c=bl),
                    in_=xb_d[g_, 0])
                d2 = nc.sync.dma_start(
                    out=xt[64:64 + C, :].rearrange("p (j c) -> p j c", c=bl),
                    in_=xb_d[g_, 1])
                add_dep_helper(d1, ms, True, "memset before x load")
                add_dep_helper(d2, ms, True, "memset before x load")
                return xt, (d1, d2)

            def scan_group(xt, xdmas, sprev):
                """Group scan -> PSUM (126, bl): sigma_even 0:63, sigma_odd
                63:117, s_next 117:126 (column-permuted host weights)."""
                sp = spsum.tile([126, bl], F32, tag="sp")
                for h in range(nh):
                    hs = slice(h * nhalf, (h + 1) * nhalf)
                    nc.tensor.matmul(sp[:, hs], lhsT=ws1h[:],
                                     rhs=sprev[:, hs],
                                     start=True, stop=False)
                    nc.tensor.matmul(sp[:, hs], lhsT=ws1h[:],
                                     rhs=sprev[:, bl + h * nhalf:
                                               bl + h * nhalf + nhalf],
                                     start=False, stop=False)
                    nc.tensor.matmul(sp[:, hs], lhsT=ws1l[:],
                                     rhs=sprev[:, hs],
                                     start=False, stop=False)
                    for j in range(NPAIR):
                        mm = nc.tensor.matmul(
                            sp[:, hs], lhsT=wqp[0:99, j * 126:(j + 1) * 126],
                            rhs=xt[0:99, j * bl + h * nhalf:
                                   j * bl + h * nhalf + nhalf],
                            start=False, stop=(j == NPAIR - 1))
                        for d in xdmas:
                            add_dep_helper(mm, d, True, "x load before scan")
                return sp

            def split_state(psum_tile):
                """psum (126, bl) f32 -> sbuf (126, 2*bl) bf16 [hi | lo].

                Both ops read PSUM (never DVE 2-port perf mode, so no shared
                SBUF port contention with GpSimd memsets); lo is produced by
                the sub directly with a bf16 output cast."""
                shl = spool.tile([126, 2 * bl], BF, tag="sprev")
                nc.vector.tensor_copy(out=shl[:, 0:bl], in_=psum_tile[:])
                last = nc.vector.tensor_sub(out=shl[:, bl:2 * bl],
                                            in0=psum_tile[:],
                                            in1=shl[:, 0:bl])
                return shl, last

            def scatter_group(xt, sprev_g, split_last):
                """sigma hi/lo -> xt rows 35:53 (even chunks), 99:117 (odd).

                4 coalesced DMAs: thanks to the r-major scan column order the
                source is a plain contiguous partition range (sigma-splitting
                source APs raced on HW); dest splits only the free dim.
                Issued from the scalar engine so they ride the ACT HWDGE ring
                ahead of the (much larger) store of the previous group.
                """
                dmas = {}
                for par in range(2):               # 0: even chunks, 1: odd
                    w = NPAIR - par
                    srows = slice(63 * par, 63 * par + 9 * w)
                    for lo in range(2):
                        d = nc.scalar.dma_start(
                            out=xt[35 + 64 * par + 9 * lo:
                                   35 + 64 * par + 9 * lo + 9,
                                   0:w * bl].rearrange("r (j c) -> r j c",
                                                       c=bl),
                            in_=sprev_g[srows, lo * bl:(lo + 1) * bl])
                        add_dep_helper(d, split_last, True,
                                       "split before scatter")
                        dmas[(par, lo)] = d
                return dmas

            def pass2_group(g_, xt, scat):
                """Per-chunk outputs into one staging tile + 1 store DMA."""
                ysb = ypool.tile([105, G * bl], F32, tag="ysb")
                copies = []
                for i in range(G):
                    jp, odd = divmod(i, 2)
                    r0 = 64 * odd
                    yp = ypsum.tile([105, bl], F32, tag="yp")
                    for h in range(nh):
                        hs = slice(jp * bl + h * nhalf,
                                   jp * bl + h * nhalf + nhalf)
                        mm = nc.tensor.matmul(
                            yp[:, h * nhalf:(h + 1) * nhalf],
                            lhsT=wmu[r0:r0 + 53, :], rhs=xt[r0:r0 + 53, hs],
                            start=True, stop=True)
                        for d in (scat[(odd, 0)], scat[(odd, 1)]):
                            add_dep_helper(mm, d, True, "scatter before pass2")
                    if i % 2 == 1:
                        cp = nc.scalar.copy(out=ysb[:, i * bl:(i + 1) * bl],
                                            in_=yp[:])
                    else:
                        cp = nc.vector.tensor_copy(
                            out=ysb[:, i * bl:(i + 1) * bl], in_=yp[:])
                    copies.append(cp)
                st = nc.scalar.dma_start(
                    out=out_d[:, G * g_:G * (g_ + 1), :],
                    in_=ysb[:].rearrange("p (k c) -> p k c", c=bl))
                for cp in copies:
                    add_dep_helper(st, cp, True, "copies before store")

            # --- init: s_0 state at rows 117:126 (zeros elsewhere)
            ip = spsum.tile([126, bl], F32, tag="sp")
            for h in range(nh):
                hs = slice(h * nhalf, (h + 1) * nhalf)
                nc.tensor.matmul(ip[:, hs], lhsT=winit[:], rhs=xinit[:, hs],
                                 start=True, stop=True)
            sprev, sprev_last = split_state(ip)

            # --- software-pipelined group loop: PE order is
            #     scan(g) ... scan(g+1), pass2(g) ... so pass2's scatter wait
            #     never stalls the (in-order) PE queue.  scatter(g+1) is also
            #     emitted before store(g) so its transfer rides the ACT HWDGE
            #     ring ahead of the big store.
            xts = {0: load_group(0), 1: load_group(1), 2: load_group(2)}
            sps = {0: scan_group(xts[0][0], xts[0][1], sprev)}
            splits = {0: split_state(sps[0])}
            scat = scatter_group(xts[0][0], splits[0][0], splits[0][1])
            for g_ in range(NG):
                if g_ + 3 < NG:
                    xts[g_ + 3] = load_group(g_ + 3)
                if g_ + 1 < NG:
                    sps[g_ + 1] = scan_group(xts[g_ + 1][0], xts[g_ + 1][1],
                                             splits[g_][0])
                    splits[g_ + 1] = split_state(sps[g_ + 1])
                    next_scat = scatter_group(xts[g_ + 1][0],
                                              splits[g_ + 1][0],
                                              splits[g_ + 1][1])
                else:
                    next_scat = None
                pass2_group(g_, xts[g_][0], scat)
                scat = next_scat
    nc.compile()
    return nc


def _prep_inputs(x, alpha, beta, gamma):
    import ml_dtypes
    bf = ml_dtypes.bfloat16
    xs = np.asarray(x, dtype=np.float32).reshape(B, L)
    coeffs = _build_coeffs(float(alpha), float(beta), float(gamma))
    in_maps = []
    for m in range(NCORES):
        xT_m = np.ascontiguousarray(xs[m * BL:(m + 1) * BL].T)  # (L, BL) f32
        xc = xT_m[1:].reshape(NG, G, C, BL).astype(bf)  # (g, chunk, step, b)
        xb = np.zeros((NG, 2, C, NPAIR, BL), bf)
        xb[:, 0] = xc[:, 0::2].transpose(0, 2, 1, 3)
        xb[:, 1, :, 0:NPAIR - 1] = xc[:, 1::2].transpose(0, 2, 1, 3)
        x0 = np.ascontiguousarray(xT_m[0:7])                    # (7, BL) f32
        in_maps.append({"xb": xb, "x0": x0, **coeffs})
    return in_maps


LAST_RESULT = None  # BassKernelResults of the most recent kernel() call


def kernel(x, alpha, beta, gamma):
    global LAST_RESULT
    from concourse.bass_utils import run_bass_kernel_spmd

    nc = build_bass(BL)
    in_maps = _prep_inputs(x, alpha, beta, gamma)
    res = run_bass_kernel_spmd(nc, in_maps, core_ids=list(range(NCORES)))
    LAST_RESULT = res
    xs = np.asarray(x, dtype=np.float32).reshape(B, L)
    y = np.empty((B, L, 3), np.float32)
    for m in range(NCORES):
        o = res.results[m]["out"]                   # (105, 117, BL) f32
        y[m * BL:(m + 1) * BL, 1:, :] = (
            o.reshape(C, 3, NG * G, BL).transpose(3, 2, 0, 1)
            .reshape(BL, L - 1, 3))
    y[:, 0, 0] = xs[:, 0]
    y[:, 0, 1] = xs[:, 1] - xs[:, 0]
    y[:, 0, 2] = 0.0
    return y


# revision 30
# speedup vs baseline: 1.3609x; 1.3609x over previous
"""Holt-Winters exponential smoothing (level/trend/seasonal, P=7) on 8 Trainium2
NeuronCores.

Math: the per-row recurrence is linear in a 9-dim state
s = [level, trend, buf_0..buf_6]:  s_t = A_{t%7} s_{t-1} + c_{t%7} x_t.
Steps t=1..4095 are processed in 117 chunks of C=35 steps (35 % 7 == 0 so every
chunk sees the same slot pattern and shares one coefficient set), grouped into
9 groups of G=13 chunks.  Chunk-entry states sigma_i come from a per-group
prefix-scan matmul; per chunk the outputs are one K=53 matmul
  Y_c (105,B) = [Wm; U; U].T @ [X_c; sig_hi; sig_lo].

Precision: x and all stationary weights are single bf16 (rel err ~2^-9, small
relative to the 2e-2 gate); only the group-to-group state chain keeps a hi/lo
bf16 split (ws1 hi/lo x state hi/lo, dropping lo*lo) since chain error
compounds over the 9 sequential groups.

Layout: chunks are paired into 128-partition tiles -- pair j holds chunk 2j at
partitions 0:35 (sigma at 35:53) and chunk 2j+1 at 64:99 (sigma at 99:117) --
so one scan matmul covers two chunks and every group's x loads with 2 fat DMAs
that together span both halves of the partition space (all 16 DMA engines).
Outputs stage into one (105, 13*1024) f32 tile per group -> 1 store DMA with
52 KiB contiguous runs per partition.

Sharding: pure data-parallel over the batch axis (1024 rows per core).
"""

import numpy as np

P = 7
C = 35            # chunk size (steps); 35 % 7 == 0
G = 13            # chunks per group
NG = 9            # groups; NG*G*C == L-1
NPAIR = 7         # chunk pairs per group (last pair has only the even chunk)
L = 4096
B = 8192
NCORES = 8
BL = B // NCORES  # 1024 batch rows per core
NHALF = 512       # matmul moving-dim tile (fp32 PSUM bank limit)


def _sigmoid(z):
    return 1.0 / (1.0 + np.exp(-z))


def _step_mats(a, b, g):
    """A_i (9x9), c_i (9,) for seasonal slot i, float64."""
    A, c = [], []
    for i in range(P):
        col = 2 + i
        Ai = np.zeros((9, 9), np.float64)
        ci = np.zeros(9, np.float64)
        Ai[0, 0] = 1 - a
        Ai[0, 1] = 1 - a
        Ai[0, col] += -a
        Ai[1, 0] = -a * b
        Ai[1, 1] = 1 - a * b
        Ai[1, col] += -a * b
        for j in range(P):
            Ai[2 + j, 2 + j] = 1.0
        Ai[col, :] = 0.0
        Ai[col, 0] = -g * (1 - a)
        Ai[col, 1] = -g * (1 - a)
        Ai[col, col] = g * a + 1 - g
        ci[0] = a
        ci[1] = a * b
        ci[col] = g * (1 - a)
        A.append(Ai)
        c.append(ci)
    return A, c


def _hi_lo(x):
    import ml_dtypes
    hi = x.astype(np.float32).astype(ml_dtypes.bfloat16)
    lo = (x.astype(np.float32) - hi.astype(np.float32)).astype(ml_dtypes.bfloat16)
    return hi, lo


def _pcol(j, r):
    """Scan-output column for component r of sigma_j.

    Component-major (r-major) layout within each parity block so the sigma
    scatter's DMA source is a plain contiguous partition range:
      sigma_even[r of pair jp] -> col 7*r + jp        (cols 0:63)
      sigma_odd [r of pair jp] -> col 63 + 6*r + jp   (cols 63:117)
      s_next[r]                -> col 117 + r
    """
    if j == G:
        return 117 + r
    jp, odd = divmod(j, 2)
    return (63 + 6 * r + jp) if odd else (7 * r + jp)


def _build_coeffs(alpha, beta, gamma):
    """Host-precomputed stationary matrices.

    wqp  (NPAIR, 128, 126) bf16: scan lhsT per chunk pair (rows 0:35 even
         chunk's X coeffs, 64:99 odd's; zeros elsewhere)
    ws1h/ws1l (126, 126) bf16: state-propagation lhsT hi/lo
    wmu  (128, 105) bf16: pass-2 lhsT [Wm; U; U] at rows 0:53 and 64:117
    winit (7, 126) f32: init matmul -> s_0 at rows 117:126
    Scan-output columns are permuted: sigma_even at 0:63, sigma_odd at 63:117,
    s_next at 117:126 (so the sigma scatter DMA is 2 dense APs).
    """
    import ml_dtypes
    bf = ml_dtypes.bfloat16
    a, b, g = _sigmoid(alpha), _sigmoid(beta), _sigmoid(gamma)
    A, c = _step_mats(a, b, g)
    slots = [(1 + k) % P for k in range(C)]

    Phi = np.zeros((C, 9, 9), np.float64)
    w = np.zeros((C, C, 9), np.float64)
    cur = np.eye(9)
    for k in range(C):
        i = slots[k]
        if k > 0:
            w[k, :k] = w[k - 1, :k] @ A[i].T
        w[k, k] = c[i]
        cur = A[i] @ cur
        Phi[k] = cur
    T = Phi[C - 1]
    V = w[C - 1].T.copy()  # (9, C)

    Wm = np.zeros((C, 105), np.float64)   # X-coefficient block of pass-2 lhsT
    U = np.zeros((9, 105), np.float64)    # sigma-coefficient block
    for k in range(C):
        sel = [0, 1, 2 + slots[k]]
        U[:, 3 * k:3 * k + 3] = Phi[k][sel].T
        for j in range(k + 1):
            Wm[j, 3 * k:3 * k + 3] = w[k, j][sel]

    Tpow = [np.eye(9)]
    for _ in range(G + 1):
        Tpow.append(T @ Tpow[-1])

    ws1 = np.zeros((126, 126), np.float64)
    for j in range(G + 1):
        for r in range(9):
            ws1[117:126, _pcol(j, r)] = Tpow[j][r, :]
    wqv = np.zeros((G, C, 126), np.float64)
    for i in range(G):
        for j in range(i + 1, G + 1):
            TV = Tpow[j - 1 - i] @ V          # (9, C)
            for r in range(9):
                wqv[i, :, _pcol(j, r)] = TV[r, :]

    winit = np.zeros((7, 126), np.float64)
    winit[0, 117] = 1.0
    winit[0, 118] = -1.0
    winit[1, 118] = 1.0
    for j in range(P):
        winit[j, 119 + j] += 1.0
        winit[0, 119 + j] += -1.0

    ws1_hi, ws1_lo = _hi_lo(ws1)

    wqp = np.zeros((NPAIR, 128, 126), bf)
    for jp in range(NPAIR):
        wqp[jp, 0:C] = wqv[2 * jp].astype(bf)
        if 2 * jp + 1 < G:
            wqp[jp, 64:64 + C] = wqv[2 * jp + 1].astype(bf)

    wmu = np.zeros((128, 105), bf)
    blk = np.concatenate([Wm, U, U], axis=0).astype(bf)   # (53, 105)
    wmu[0:53] = blk
    wmu[64:117] = blk

    return dict(wqp=wqp, ws1h=ws1_hi, ws1l=ws1_lo, wmu=wmu,
                winit=winit.astype(np.float32))


def build_bass(bl=BL):
    """Build the per-core Bass module (SPMD: same module, sharded inputs)."""
    import concourse.bacc as bacc
    import concourse.mybir as mybir
    from concourse.tile import TileContext

    BF = mybir.dt.bfloat16
    F32 = mybir.dt.float32
    nhalf = min(NHALF, bl)
    nh = (bl + nhalf - 1) // nhalf

    nc = bacc.Bacc(None, target_bir_lowering=False, debug=False)
    xb_d = nc.declare_dram_parameter("xb", [NG, 2, C, NPAIR, bl], BF,
                                     isOutput=False)
    x0_d = nc.declare_dram_parameter("x0", [7, bl], F32, isOutput=False)
    wqp_d = nc.declare_dram_parameter("wqp", [NPAIR, 128, 126], BF,
                                      isOutput=False)
    ws1h_d = nc.declare_dram_parameter("ws1h", [126, 126], BF, isOutput=False)
    ws1l_d = nc.declare_dram_parameter("ws1l", [126, 126], BF, isOutput=False)
    wmu_d = nc.declare_dram_parameter("wmu", [128, 105], BF, isOutput=False)
    winit_d = nc.declare_dram_parameter("winit", [7, 126], F32, isOutput=False)
    # [p, chunk-in-group, group, batch]: the store's per-partition runs are
    # then 4 KiB (not 52 KiB), so the SDMA engines' packet round-robin shares
    # bandwidth fairly between the store queue and the load/scatter queue.
    out_d = nc.declare_dram_parameter("out", [105, G, NG, bl], F32,
                                      isOutput=True)

    from concourse.tile_rust import add_dep_helper as _adh

    def add_dep_helper(frm, to, sync=True, reason=""):
        frm = getattr(frm, "ins", frm)
        to = getattr(to, "ins", to)
        _adh(frm, to, sync, reason)

    with TileContext(nc) as tc:
        with (
            tc.tile_pool(name="consts", bufs=1) as consts,
            tc.tile_pool(name="xpool", bufs=4) as xpool,
            tc.tile_pool(name="spool", bufs=3) as spool,
            tc.tile_pool(name="tpool", bufs=2) as tpool,
            tc.tile_pool(name="ypool", bufs=2) as ypool,
            tc.tile_pool(name="ypsum", bufs=3, space="PSUM") as ypsum,
            tc.tile_pool(name="spsum", bufs=1, space="PSUM") as spsum,
        ):
            wqp = consts.tile([128, NPAIR * 126], BF)
            for j in range(NPAIR):
                nc.sync.dma_start(out=wqp[:, j * 126:(j + 1) * 126],
                                  in_=wqp_d[j])
            ws1h = consts.tile([126, 126], BF)
            nc.sync.dma_start(out=ws1h[:], in_=ws1h_d[:])
            ws1l = consts.tile([126, 126], BF)
            nc.sync.dma_start(out=ws1l[:], in_=ws1l_d[:])
            wmu = consts.tile([128, 105], BF)
            nc.scalar.dma_start(out=wmu[:], in_=wmu_d[:])
            winit = consts.tile([7, 126], F32)
            nc.scalar.dma_start(out=winit[:], in_=winit_d[:])
            xinit = consts.tile([7, bl], F32)
            nc.scalar.dma_start(out=xinit[:], in_=x0_d[:])

            def load_group(g_):
                """x load for group g_: memset NaN-guard rows, then 2 fat
                DMAs (parts 0:35 even band, 64:99 odd band)."""
                xt = xpool.tile([128, NPAIR * bl], BF, tag="xg")
                # zero sigma+pad rows 32:64 (rows 53:64 are read by the scan
                # with zero weights and never DMA-written; must be finite)
                ms = nc.gpsimd.memset(xt[32:64, :], 0.0)
                d1 = nc.sync.dma_start(
                    out=xt[0:C, :].rearrange("p (j c) -> p j c", c=bl),
                    in_=xb_d[g_, 0])
                d2 = nc.sync.dma_start(
                    out=xt[64:64 + C, :].rearrange("p (j c) -> p j c", c=bl),
                    in_=xb_d[g_, 1])
                add_dep_helper(d1, ms, True, "memset before x load")
                add_dep_helper(d2, ms, True, "memset before x load")
                return xt, (d1, d2)

            def scan_group(xt, xdmas, sprev):
                """Group scan -> PSUM (126, bl): sigma_even 0:63, sigma_odd
                63:117, s_next 117:126 (column-permuted host weights)."""
                sp = spsum.tile([126, bl], F32, tag="sp")
                for h in range(nh):
                    hs = slice(h * nhalf, (h + 1) * nhalf)
                    nc.tensor.matmul(sp[:, hs], lhsT=ws1h[:],
                                     rhs=sprev[:, hs],
                                     start=True, stop=False)
                    nc.tensor.matmul(sp[:, hs], lhsT=ws1h[:],
                                     rhs=sprev[:, bl + h * nhalf:
                                               bl + h * nhalf + nhalf],
                                     start=False, stop=False)
                    nc.tensor.matmul(sp[:, hs], lhsT=ws1l[:],
                                     rhs=sprev[:, hs],
                                     start=False, stop=False)
                    for j in range(NPAIR):
                        mm = nc.tensor.matmul(
                            sp[:, hs], lhsT=wqp[0:99, j * 126:(j + 1) * 126],
                            rhs=xt[0:99, j * bl + h * nhalf:
                                   j * bl + h * nhalf + nhalf],
                            start=False, stop=(j == NPAIR - 1))
                        for d in xdmas:
                            add_dep_helper(mm, d, True, "x load before scan")
                return sp

            def split_state(psum_tile):
                """psum (126, bl) f32 -> sbuf (126, 2*bl) bf16 [hi | lo].

                Both ops read PSUM (never DVE 2-port perf mode, so no shared
                SBUF port contention with GpSimd memsets); lo is produced by
                the sub directly with a bf16 output cast."""
                shl = spool.tile([126, 2 * bl], BF, tag="sprev")
                nc.vector.tensor_copy(out=shl[:, 0:bl], in_=psum_tile[:])
                last = nc.vector.tensor_sub(out=shl[:, bl:2 * bl],
                                            in0=psum_tile[:],
                                            in1=shl[:, 0:bl])
                return shl, last

            def scatter_group(xt, sprev_g, split_last):
                """sigma hi/lo -> xt rows 35:53 (even chunks), 99:117 (odd).

                4 coalesced DMAs: thanks to the r-major scan column order the
                source is a plain contiguous partition range (sigma-splitting
                source APs raced on HW); dest splits only the free dim.
                """
                dmas = {}
                for par in range(2):               # 0: even chunks, 1: odd
                    w = NPAIR - par
                    srows = slice(63 * par, 63 * par + 9 * w)
                    for lo in range(2):
                        d = nc.sync.dma_start(
                            out=xt[35 + 64 * par + 9 * lo:
                                   35 + 64 * par + 9 * lo + 9,
                                   0:w * bl].rearrange("r (j c) -> r j c",
                                                       c=bl),
                            in_=sprev_g[srows, lo * bl:(lo + 1) * bl])
                        add_dep_helper(d, split_last, True,
                                       "split before scatter")
                        dmas[(par, lo)] = d
                return dmas

            def pass2_group(g_, xt, scat):
                """Per-chunk outputs into one staging tile + 1 store DMA."""
                ysb = ypool.tile([105, G * bl], F32, tag="ysb")
                copies = []
                for i in range(G):
                    jp, odd = divmod(i, 2)
                    r0 = 64 * odd
                    yp = ypsum.tile([105, bl], F32, tag="yp")
                    for h in range(nh):
                        hs = slice(jp * bl + h * nhalf,
                                   jp * bl + h * nhalf + nhalf)
                        mm = nc.tensor.matmul(
                            yp[:, h * nhalf:(h + 1) * nhalf],
                            lhsT=wmu[r0:r0 + 53, :], rhs=xt[r0:r0 + 53, hs],
                            start=True, stop=True)
                        for d in (scat[(odd, 0)], scat[(odd, 1)]):
                            add_dep_helper(mm, d, True, "scatter before pass2")
                    if i % 2 == 1:
                        cp = nc.scalar.copy(out=ysb[:, i * bl:(i + 1) * bl],
                                            in_=yp[:])
                    else:
                        cp = nc.vector.tensor_copy(
                            out=ysb[:, i * bl:(i + 1) * bl], in_=yp[:])
                    copies.append(cp)
                st = nc.scalar.dma_start(
                    out=out_d[:, :, g_, :],
                    in_=ysb[:].rearrange("p (k c) -> p k c", c=bl))
                for cp in copies:
                    add_dep_helper(st, cp, True, "copies before store")

            # --- init: s_0 state at rows 117:126 (zeros elsewhere)
            ip = spsum.tile([126, bl], F32, tag="sp")
            for h in range(nh):
                hs = slice(h * nhalf, (h + 1) * nhalf)
                nc.tensor.matmul(ip[:, hs], lhsT=winit[:], rhs=xinit[:, hs],
                                 start=True, stop=True)
            sprev, sprev_last = split_state(ip)

            # --- software-pipelined group loop: PE order is
            #     scan(g) ... scan(g+1), pass2(g) ... so pass2's scatter wait
            #     never stalls the (in-order) PE queue.  scatter(g+1) is also
            #     emitted before store(g) so its transfer rides the ACT HWDGE
            #     ring ahead of the big store.
            xts = {0: load_group(0), 1: load_group(1), 2: load_group(2)}
            sps = {0: scan_group(xts[0][0], xts[0][1], sprev)}
            splits = {0: split_state(sps[0])}
            scat = scatter_group(xts[0][0], splits[0][0], splits[0][1])
            for g_ in range(NG):
                if g_ + 3 < NG:
                    xts[g_ + 3] = load_group(g_ + 3)
                if g_ + 1 < NG:
                    sps[g_ + 1] = scan_group(xts[g_ + 1][0], xts[g_ + 1][1],
                                             splits[g_][0])
                    splits[g_ + 1] = split_state(sps[g_ + 1])
                    next_scat = scatter_group(xts[g_ + 1][0],
                                              splits[g_ + 1][0],
                                              splits[g_ + 1][1])
                else:
                    next_scat = None
                pass2_group(g_, xts[g_][0], scat)
                scat = next_scat
    nc.compile()
    return nc


def _prep_inputs(x, alpha, beta, gamma):
    import ml_dtypes
    bf = ml_dtypes.bfloat16
    xs = np.asarray(x, dtype=np.float32).reshape(B, L)
    coeffs = _build_coeffs(float(alpha), float(beta), float(gamma))
    in_maps = []
    for m in range(NCORES):
        xT_m = np.ascontiguousarray(xs[m * BL:(m + 1) * BL].T)  # (L, BL) f32
        xc = xT_m[1:].reshape(NG, G, C, BL).astype(bf)  # (g, chunk, step, b)
        xb = np.zeros((NG, 2, C, NPAIR, BL), bf)
        xb[:, 0] = xc[:, 0::2].transpose(0, 2, 1, 3)
        xb[:, 1, :, 0:NPAIR - 1] = xc[:, 1::2].transpose(0, 2, 1, 3)
        x0 = np.ascontiguousarray(xT_m[0:7])                    # (7, BL) f32
        in_maps.append({"xb": xb, "x0": x0, **coeffs})
    return in_maps


LAST_RESULT = None  # BassKernelResults of the most recent kernel() call


def kernel(x, alpha, beta, gamma):
    global LAST_RESULT
    from concourse.bass_utils import run_bass_kernel_spmd

    nc = build_bass(BL)
    in_maps = _prep_inputs(x, alpha, beta, gamma)
    res = run_bass_kernel_spmd(nc, in_maps, core_ids=list(range(NCORES)))
    LAST_RESULT = res
    xs = np.asarray(x, dtype=np.float32).reshape(B, L)
    y = np.empty((B, L, 3), np.float32)
    for m in range(NCORES):
        o = res.results[m]["out"]                   # (105, G, NG, BL) f32
        y[m * BL:(m + 1) * BL, 1:, :] = (
            o.reshape(C, 3, G, NG, BL).transpose(4, 3, 2, 0, 1)
            .reshape(BL, L - 1, 3))
    y[:, 0, 0] = xs[:, 0]
    y[:, 0, 1] = xs[:, 1] - xs[:, 0]
    y[:, 0, 2] = 0.0
    return y


# revision 33
# speedup vs baseline: 1.4266x; 1.0482x over previous
"""Holt-Winters exponential smoothing (level/trend/seasonal, P=7) on 8 Trainium2
NeuronCores.

Math: the per-row recurrence is linear in a 9-dim state
s = [level, trend, buf_0..buf_6]:  s_t = A_{t%7} s_{t-1} + c_{t%7} x_t.
Steps t=1..4095 are processed in 117 chunks of C=35 steps (35 % 7 == 0 so every
chunk sees the same slot pattern and shares one coefficient set), grouped into
9 groups of G=13 chunks.  Chunk-entry states sigma_i come from a per-group
prefix-scan matmul; per chunk the outputs are one K=53 matmul
  Y_c (105,B) = [Wm; U; U].T @ [X_c; sig_hi; sig_lo].

Precision: x and all stationary weights are single bf16 (rel err ~2^-9, small
relative to the 2e-2 gate); only the group-to-group state chain keeps a hi/lo
bf16 split (ws1 hi/lo x state hi/lo, dropping lo*lo) since chain error
compounds over the 9 sequential groups.

Layout: chunks are paired into 128-partition tiles -- pair j holds chunk 2j at
partitions 0:35 (sigma at 35:53) and chunk 2j+1 at 64:99 (sigma at 99:117) --
so one scan matmul covers two chunks and every group's x loads with 2 fat DMAs
that together span both halves of the partition space (all 16 DMA engines).
Outputs stage into one (105, 13*1024) f32 tile per group -> 1 store DMA with
52 KiB contiguous runs per partition.

Sharding: pure data-parallel over the batch axis (1024 rows per core).
"""

import numpy as np

P = 7
C = 35            # chunk size (steps); 35 % 7 == 0
G = 13            # chunks per group
NG = 9            # groups; NG*G*C == L-1
NPAIR = 7         # chunk pairs per group (last pair has only the even chunk)
L = 4096
B = 8192
NCORES = 8
BL = B // NCORES  # 1024 batch rows per core
NHALF = 512       # matmul moving-dim tile (fp32 PSUM bank limit)


def _sigmoid(z):
    return 1.0 / (1.0 + np.exp(-z))


def _step_mats(a, b, g):
    """A_i (9x9), c_i (9,) for seasonal slot i, float64."""
    A, c = [], []
    for i in range(P):
        col = 2 + i
        Ai = np.zeros((9, 9), np.float64)
        ci = np.zeros(9, np.float64)
        Ai[0, 0] = 1 - a
        Ai[0, 1] = 1 - a
        Ai[0, col] += -a
        Ai[1, 0] = -a * b
        Ai[1, 1] = 1 - a * b
        Ai[1, col] += -a * b
        for j in range(P):
            Ai[2 + j, 2 + j] = 1.0
        Ai[col, :] = 0.0
        Ai[col, 0] = -g * (1 - a)
        Ai[col, 1] = -g * (1 - a)
        Ai[col, col] = g * a + 1 - g
        ci[0] = a
        ci[1] = a * b
        ci[col] = g * (1 - a)
        A.append(Ai)
        c.append(ci)
    return A, c


def _hi_lo(x):
    import ml_dtypes
    hi = x.astype(np.float32).astype(ml_dtypes.bfloat16)
    lo = (x.astype(np.float32) - hi.astype(np.float32)).astype(ml_dtypes.bfloat16)
    return hi, lo


def _pcol(j, r):
    """Scan-output column for component r of sigma_j.

    Component-major (r-major) layout within each parity block so the sigma
    scatter's DMA source is a plain contiguous partition range:
      sigma_even[r of pair jp] -> col 7*r + jp        (cols 0:63)
      sigma_odd [r of pair jp] -> col 63 + 6*r + jp   (cols 63:117)
      s_next[r]                -> col 117 + r
    """
    if j == G:
        return 117 + r
    jp, odd = divmod(j, 2)
    return (63 + 6 * r + jp) if odd else (7 * r + jp)


def _build_coeffs(alpha, beta, gamma):
    """Host-precomputed stationary matrices.

    wqp  (NPAIR, 128, 126) bf16: scan lhsT per chunk pair (rows 0:35 even
         chunk's X coeffs, 64:99 odd's; zeros elsewhere)
    ws1h/ws1l (126, 126) bf16: state-propagation lhsT hi/lo
    wmu  (128, 105) bf16: pass-2 lhsT [Wm; U; U] at rows 0:53 and 64:117
    winit (7, 126) f32: init matmul -> s_0 at rows 117:126
    Scan-output columns are permuted: sigma_even at 0:63, sigma_odd at 63:117,
    s_next at 117:126 (so the sigma scatter DMA is 2 dense APs).
    """
    import ml_dtypes
    bf = ml_dtypes.bfloat16
    a, b, g = _sigmoid(alpha), _sigmoid(beta), _sigmoid(gamma)
    A, c = _step_mats(a, b, g)
    slots = [(1 + k) % P for k in range(C)]

    Phi = np.zeros((C, 9, 9), np.float64)
    w = np.zeros((C, C, 9), np.float64)
    cur = np.eye(9)
    for k in range(C):
        i = slots[k]
        if k > 0:
            w[k, :k] = w[k - 1, :k] @ A[i].T
        w[k, k] = c[i]
        cur = A[i] @ cur
        Phi[k] = cur
    T = Phi[C - 1]
    V = w[C - 1].T.copy()  # (9, C)

    Wm = np.zeros((C, 105), np.float64)   # X-coefficient block of pass-2 lhsT
    U = np.zeros((9, 105), np.float64)    # sigma-coefficient block
    for k in range(C):
        sel = [0, 1, 2 + slots[k]]
        U[:, 3 * k:3 * k + 3] = Phi[k][sel].T
        for j in range(k + 1):
            Wm[j, 3 * k:3 * k + 3] = w[k, j][sel]

    Tpow = [np.eye(9)]
    for _ in range(G + 1):
        Tpow.append(T @ Tpow[-1])

    ws1 = np.zeros((126, 126), np.float64)
    for j in range(G + 1):
        for r in range(9):
            ws1[117:126, _pcol(j, r)] = Tpow[j][r, :]
    wqv = np.zeros((G, C, 126), np.float64)
    for i in range(G):
        for j in range(i + 1, G + 1):
            TV = Tpow[j - 1 - i] @ V          # (9, C)
            for r in range(9):
                wqv[i, :, _pcol(j, r)] = TV[r, :]

    winit = np.zeros((7, 126), np.float64)
    winit[0, 117] = 1.0
    winit[0, 118] = -1.0
    winit[1, 118] = 1.0
    for j in range(P):
        winit[j, 119 + j] += 1.0
        winit[0, 119 + j] += -1.0

    ws1_hi, ws1_lo = _hi_lo(ws1)

    wqp = np.zeros((NPAIR, 128, 126), bf)
    for jp in range(NPAIR):
        wqp[jp, 0:C] = wqv[2 * jp].astype(bf)
        if 2 * jp + 1 < G:
            wqp[jp, 64:64 + C] = wqv[2 * jp + 1].astype(bf)

    wmu = np.zeros((128, 105), bf)
    blk = np.concatenate([Wm, U, U], axis=0).astype(bf)   # (53, 105)
    wmu[0:53] = blk
    wmu[64:117] = blk

    return dict(wqp=wqp, ws1h=ws1_hi, ws1l=ws1_lo, wmu=wmu,
                winit=winit.astype(np.float32))


def build_bass(bl=BL):
    """Build the per-core Bass module (SPMD: same module, sharded inputs)."""
    import concourse.bacc as bacc
    import concourse.mybir as mybir
    from concourse.tile import TileContext

    BF = mybir.dt.bfloat16
    F32 = mybir.dt.float32
    nhalf = min(NHALF, bl)
    nh = (bl + nhalf - 1) // nhalf

    nc = bacc.Bacc(None, target_bir_lowering=False, debug=False)
    xb_d = nc.declare_dram_parameter("xb", [NG, 2, C, NPAIR, bl], BF,
                                     isOutput=False)
    x0_d = nc.declare_dram_parameter("x0", [7, bl], F32, isOutput=False)
    wqp_d = nc.declare_dram_parameter("wqp", [NPAIR, 128, 126], BF,
                                      isOutput=False)
    ws1h_d = nc.declare_dram_parameter("ws1h", [126, 126], BF, isOutput=False)
    ws1l_d = nc.declare_dram_parameter("ws1l", [126, 126], BF, isOutput=False)
    wmu_d = nc.declare_dram_parameter("wmu", [128, 105], BF, isOutput=False)
    winit_d = nc.declare_dram_parameter("winit", [7, 126], F32, isOutput=False)
    # [p, chunk-in-group, group, batch]: the store's per-partition runs are
    # then 4 KiB (not 52 KiB), so the SDMA engines' packet round-robin shares
    # bandwidth fairly between the store queue and the load/scatter queue.
    out_d = nc.declare_dram_parameter("out", [105, G, NG, bl], F32,
                                      isOutput=True)

    from concourse.tile_rust import add_dep_helper as _adh

    def add_dep_helper(frm, to, sync=True, reason=""):
        frm = getattr(frm, "ins", frm)
        to = getattr(to, "ins", to)
        _adh(frm, to, sync, reason)

    with TileContext(nc) as tc:
        with (
            tc.tile_pool(name="consts", bufs=1) as consts,
            tc.tile_pool(name="xpool", bufs=4) as xpool,
            tc.tile_pool(name="spool", bufs=3) as spool,
            tc.tile_pool(name="tpool", bufs=2) as tpool,
            tc.tile_pool(name="ypool", bufs=2) as ypool,
            tc.tile_pool(name="ypsum", bufs=4, space="PSUM") as ypsum,
            tc.tile_pool(name="spsum", bufs=2, space="PSUM") as spsum,
        ):
            wqp = consts.tile([128, NPAIR * 126], BF)
            for j in range(NPAIR):
                nc.sync.dma_start(out=wqp[:, j * 126:(j + 1) * 126],
                                  in_=wqp_d[j])
            ws1h = consts.tile([126, 126], BF)
            nc.sync.dma_start(out=ws1h[:], in_=ws1h_d[:])
            ws1l = consts.tile([126, 126], BF)
            nc.sync.dma_start(out=ws1l[:], in_=ws1l_d[:])
            wmu = consts.tile([128, 105], BF)
            nc.scalar.dma_start(out=wmu[:], in_=wmu_d[:])
            winit = consts.tile([7, 126], F32)
            nc.scalar.dma_start(out=winit[:], in_=winit_d[:])
            xinit = consts.tile([7, bl], F32)
            nc.scalar.dma_start(out=xinit[:], in_=x0_d[:])

            def load_group(g_):
                """x load for group g_: memset NaN-guard rows, then 2 fat
                DMAs (parts 0:35 even band, 64:99 odd band)."""
                xt = xpool.tile([128, NPAIR * bl], BF, tag="xg")
                # zero sigma+pad rows 32:64 (rows 53:64 are read by the scan
                # with zero weights and never DMA-written; must be finite)
                ms = nc.gpsimd.memset(xt[32:64, :], 0.0)
                d1 = nc.sync.dma_start(
                    out=xt[0:C, :].rearrange("p (j c) -> p j c", c=bl),
                    in_=xb_d[g_, 0])
                d2 = nc.sync.dma_start(
                    out=xt[64:64 + C, :].rearrange("p (j c) -> p j c", c=bl),
                    in_=xb_d[g_, 1])
                add_dep_helper(d1, ms, True, "memset before x load")
                add_dep_helper(d2, ms, True, "memset before x load")
                return xt, (d1, d2)

            def scan_group(xt, xdmas, sprev):
                """Group scan -> PSUM (126, bl): sigma_even 0:63, sigma_odd
                63:117, s_next 117:126 (column-permuted host weights)."""
                sp = spsum.tile([126, bl], F32, tag="sp")
                for h in range(nh):
                    hs = slice(h * nhalf, (h + 1) * nhalf)
                    # pair MMs first: they only need the x tile, so the PE
                    # can start them before split(g) has produced sprev
                    for j in range(NPAIR):
                        mm = nc.tensor.matmul(
                            sp[:, hs], lhsT=wqp[0:99, j * 126:(j + 1) * 126],
                            rhs=xt[0:99, j * bl + h * nhalf:
                                   j * bl + h * nhalf + nhalf],
                            start=(j == 0), stop=False)
                        for d in xdmas:
                            add_dep_helper(mm, d, True, "x load before scan")
                    nc.tensor.matmul(sp[:, hs], lhsT=ws1h[:],
                                     rhs=sprev[:, hs],
                                     start=False, stop=False)
                    nc.tensor.matmul(sp[:, hs], lhsT=ws1h[:],
                                     rhs=sprev[:, bl + h * nhalf:
                                               bl + h * nhalf + nhalf],
                                     start=False, stop=False)
                    nc.tensor.matmul(sp[:, hs], lhsT=ws1l[:],
                                     rhs=sprev[:, hs],
                                     start=False, stop=True)
                return sp

            def split_state(psum_tile):
                """psum (126, bl) f32 -> sbuf (126, 2*bl) bf16 [hi | lo].

                Both ops read PSUM (never DVE 2-port perf mode, so no shared
                SBUF port contention with GpSimd memsets); lo is produced by
                the sub directly with a bf16 output cast."""
                shl = spool.tile([126, 2 * bl], BF, tag="sprev")
                nc.vector.tensor_copy(out=shl[:, 0:bl], in_=psum_tile[:])
                last = nc.vector.tensor_sub(out=shl[:, bl:2 * bl],
                                            in0=psum_tile[:],
                                            in1=shl[:, 0:bl])
                return shl, last

            def scatter_group(xt, sprev_g, split_last):
                """sigma hi/lo -> xt rows 35:53 (even chunks), 99:117 (odd).

                4 coalesced DMAs: thanks to the r-major scan column order the
                source is a plain contiguous partition range (sigma-splitting
                source APs raced on HW); dest splits only the free dim.
                """
                dmas = {}
                for par in range(2):               # 0: even chunks, 1: odd
                    w = NPAIR - par
                    srows = slice(63 * par, 63 * par + 9 * w)
                    for lo in range(2):
                        d = nc.sync.dma_start(
                            out=xt[35 + 64 * par + 9 * lo:
                                   35 + 64 * par + 9 * lo + 9,
                                   0:w * bl].rearrange("r (j c) -> r j c",
                                                       c=bl),
                            in_=sprev_g[srows, lo * bl:(lo + 1) * bl])
                        add_dep_helper(d, split_last, True,
                                       "split before scatter")
                        dmas[(par, lo)] = d
                return dmas

            def pass2_group(g_, xt, scat):
                """Per-chunk outputs into one staging tile + 1 store DMA."""
                ysb = ypool.tile([105, G * bl], F32, tag="ysb")
                copies = []
                for i in range(G):
                    jp, odd = divmod(i, 2)
                    r0 = 64 * odd
                    for h in range(nh):
                        yp = ypsum.tile([105, nhalf], F32, tag="yp")
                        hs = slice(jp * bl + h * nhalf,
                                   jp * bl + h * nhalf + nhalf)
                        mm = nc.tensor.matmul(
                            yp[:], lhsT=wmu[r0:r0 + 53, :],
                            rhs=xt[r0:r0 + 53, hs], start=True, stop=True)
                        for d in (scat[(odd, 0)], scat[(odd, 1)]):
                            add_dep_helper(mm, d, True, "scatter before pass2")
                        ds = slice(i * bl + h * nhalf,
                                   i * bl + (h + 1) * nhalf)
                        if (2 * i + h) % 2:
                            cp = nc.scalar.copy(out=ysb[:, ds], in_=yp[:])
                        else:
                            cp = nc.vector.tensor_copy(out=ysb[:, ds],
                                                       in_=yp[:])
                        copies.append(cp)
                st = nc.scalar.dma_start(
                    out=out_d[:, :, g_, :],
                    in_=ysb[:].rearrange("p (k c) -> p k c", c=bl))
                for cp in copies:
                    add_dep_helper(st, cp, True, "copies before store")

            # --- init: s_0 state at rows 117:126 (zeros elsewhere)
            ip = spsum.tile([126, bl], F32, tag="sp")
            for h in range(nh):
                hs = slice(h * nhalf, (h + 1) * nhalf)
                nc.tensor.matmul(ip[:, hs], lhsT=winit[:], rhs=xinit[:, hs],
                                 start=True, stop=True)
            sprev, sprev_last = split_state(ip)

            # --- software-pipelined group loop: PE order is
            #     scan(g) ... scan(g+1), pass2(g) ... so pass2's scatter wait
            #     never stalls the (in-order) PE queue.  scatter(g+1) is also
            #     emitted before store(g) so its transfer rides the ACT HWDGE
            #     ring ahead of the big store.
            xts = {0: load_group(0), 1: load_group(1), 2: load_group(2)}
            sps = {0: scan_group(xts[0][0], xts[0][1], sprev)}
            splits = {0: split_state(sps[0])}
            scat = scatter_group(xts[0][0], splits[0][0], splits[0][1])
            for g_ in range(NG):
                if g_ + 3 < NG:
                    xts[g_ + 3] = load_group(g_ + 3)
                if g_ + 1 < NG:
                    sps[g_ + 1] = scan_group(xts[g_ + 1][0], xts[g_ + 1][1],
                                             splits[g_][0])
                    splits[g_ + 1] = split_state(sps[g_ + 1])
                    next_scat = scatter_group(xts[g_ + 1][0],
                                              splits[g_ + 1][0],
                                              splits[g_ + 1][1])
                else:
                    next_scat = None
                pass2_group(g_, xts[g_][0], scat)
                scat = next_scat
    nc.compile()
    return nc


def _prep_inputs(x, alpha, beta, gamma):
    import ml_dtypes
    bf = ml_dtypes.bfloat16
    xs = np.asarray(x, dtype=np.float32).reshape(B, L)
    coeffs = _build_coeffs(float(alpha), float(beta), float(gamma))
    in_maps = []
    for m in range(NCORES):
        xT_m = np.ascontiguousarray(xs[m * BL:(m + 1) * BL].T)  # (L, BL) f32
        xc = xT_m[1:].reshape(NG, G, C, BL).astype(bf)  # (g, chunk, step, b)
        xb = np.zeros((NG, 2, C, NPAIR, BL), bf)
        xb[:, 0] = xc[:, 0::2].transpose(0, 2, 1, 3)
        xb[:, 1, :, 0:NPAIR - 1] = xc[:, 1::2].transpose(0, 2, 1, 3)
        x0 = np.ascontiguousarray(xT_m[0:7])                    # (7, BL) f32
        in_maps.append({"xb": xb, "x0": x0, **coeffs})
    return in_maps


LAST_RESULT = None  # BassKernelResults of the most recent kernel() call


def kernel(x, alpha, beta, gamma):
    global LAST_RESULT
    from concourse.bass_utils import run_bass_kernel_spmd

    nc = build_bass(BL)
    in_maps = _prep_inputs(x, alpha, beta, gamma)
    res = run_bass_kernel_spmd(nc, in_maps, core_ids=list(range(NCORES)))
    LAST_RESULT = res
    xs = np.asarray(x, dtype=np.float32).reshape(B, L)
    y = np.empty((B, L, 3), np.float32)
    for m in range(NCORES):
        o = res.results[m]["out"]                   # (105, G, NG, BL) f32
        y[m * BL:(m + 1) * BL, 1:, :] = (
            o.reshape(C, 3, G, NG, BL).transpose(4, 3, 2, 0, 1)
            .reshape(BL, L - 1, 3))
    y[:, 0, 0] = xs[:, 0]
    y[:, 0, 1] = xs[:, 1] - xs[:, 0]
    y[:, 0, 2] = 0.0
    return y
